# revision 3
# baseline (speedup 1.0000x reference)
"""Block-sparse top-k masked linear for Trainium2, tensor-parallel over 8 cores.

out = (block_masked x) @ W + bias
  x: (128, 1, 4096) fp16, W: (4096, 11008) fp16, bias: (11008,) fp16
  mask: per (32-row x 64-col) block of x, keep blocks whose mean |x| is
  >= the 32nd-largest of the 64 k-block activations in that row block.

Sharding: column-parallel — each of the 8 cores gets an 11008/8 = 1376
column slice of W and bias; x is replicated; outputs are concatenated.
"""
from contextlib import ExitStack

import numpy as np

import concourse.bass as bass
import concourse.tile as tile
from concourse import bacc, mybir
from concourse.bass_utils import run_bass_kernel_spmd

F16 = mybir.dt.float16
F32 = mybir.dt.float32
AX = mybir.AxisListType
ALU = mybir.AluOpType
ACT = mybir.ActivationFunctionType

M = 128          # rows of x
K = 4096         # contraction
N = 11008        # out features
NCORES = 8
NLOC = N // NCORES           # 1376 columns per core
BLOCK_M, BLOCK_K = 32, 64
NBM, NBK = M // BLOCK_M, K // BLOCK_K   # 4 row blocks, 64 k blocks
KEEP = 32                               # k blocks kept per row block
NKT = K // 128                          # 32 k tiles of 128
N_TILES = [(0, 512), (512, 512), (1024, 352)]   # n-tile offsets/sizes
W_BUFS = 6


def _program(ctx: ExitStack, tc: tile.TileContext, ins, outs):
    nc = tc.nc
    x_d, w_d, b_d, e_d, selh_d = ins
    (o_d,) = outs

    const = ctx.enter_context(tc.tile_pool(name="const", bufs=1))
    xbuf = ctx.enter_context(tc.tile_pool(name="xbuf", bufs=1))
    mk = ctx.enter_context(tc.tile_pool(name="mk", bufs=1))
    wpool = ctx.enter_context(tc.tile_pool(name="wpool", bufs=W_BUFS))
    opool = ctx.enter_context(tc.tile_pool(name="opool", bufs=1))
    psum = ctx.enter_context(tc.tile_pool(name="psum", bufs=1, space="PSUM"))
    dram = ctx.enter_context(tc.tile_pool(name="dram", bufs=1, space="DRAM"))

    # ---- mask pipeline (natural x layout; independent of the transposes) ----
    # x_sb[m, k], loaded in 4 column chunks so the reduce can start early
    x_sb = xbuf.tile([128, K], F16)
    part_n = mk.tile([128, NBK], F32)
    for c in range(4):
        ks = K // 4
        nc.sync.dma_start(x_sb[:, c * ks:(c + 1) * ks], x_d[:, c * ks:(c + 1) * ks])
        # part_n[m, j] = sum_k |x[m, 64 j + k]| over the 16 j's in this chunk
        nc.vector.tensor_reduce(
            part_n[:, c * 16:(c + 1) * 16],
            x_sb[:, c * ks:(c + 1) * ks].rearrange("p (j k) -> p j k", k=BLOCK_K),
            axis=AX.X, op=ALU.add, apply_absolute_value=True)

    # ba_ps[b, j] = sum_m E[m, b] * part_n[m, j]  (block sums, b on partitions)
    e_sb = const.tile([128, NBM], F32)
    nc.sync.dma_start(e_sb[:], e_d)
    ba_ps = psum.tile([NBM, NBK], F32, tag="ba_ps")
    nc.tensor.matmul(ba_ps[:], lhsT=e_sb[:], rhs=part_n[:], start=True, stop=True)

    # mean = sum / 2048 (exact power of two), rounded to f16 like jnp.mean
    ba16 = mk.tile([NBM, NBK], F16)
    nc.scalar.activation(ba16[:], ba_ps[:], ACT.Copy, scale=1.0 / 2048.0)

    # DRAM round trip #1: replicate/transpose the 4x64 activations
    ba_dram = dram.tile([NBM, NBK], F16)
    nc.sync.dma_start(ba_dram[:], ba16[:])
    # arow[i, b*64 + j] = a[b, j]  (same 256 values on all 64 partitions)
    arow = mk.tile([64, NBM * NBK], F16)
    nc.sync.dma_start(
        arow[:].rearrange("i (b j) -> i b j", b=NBM),
        ba_dram[:].unsqueeze(0).broadcast_to((64, NBM, NBK)))
    # acol[i, b] = a[b, i]
    acol = mk.tile([64, NBM], F16)
    nc.sync.dma_start(acol[:], ba_dram[:].rearrange("b j -> j b"))

    # cnt[i, b] = #{j : a[b, j] > a[b, i]};  keep iff cnt < KEEP
    cmp = mk.tile([64, NBM * NBK], F16)
    nc.vector.tensor_tensor(
        cmp[:].rearrange("i (b j) -> i b j", b=NBM),
        arow[:].rearrange("i (b j) -> i b j", b=NBM),
        acol[:].unsqueeze(-1).broadcast_to((64, NBM, NBK)),
        op=ALU.is_gt)
    cnt = mk.tile([64, NBM], F32)
    nc.vector.tensor_reduce(cnt[:], cmp[:].rearrange("i (b j) -> i b j", b=NBM),
                            axis=AX.X, op=ALU.add)
    keep16 = mk.tile([64, NBM], F16)
    nc.vector.tensor_scalar(keep16[:], cnt[:], float(KEEP), None, op0=ALU.is_lt)

    # DRAM round trip #2: keeph[h, b*32 + kt] = keep16[2 kt + h, b]
    keep_dram = dram.tile([64, NBM], F16)
    nc.sync.dma_start(keep_dram[:], keep16[:])
    keeph = mk.tile([2, 128], F16)
    nc.sync.dma_start(keeph[:].rearrange("h (b kt) -> h b kt", b=NBM),
                      keep_dram[:].rearrange("(kt h) b -> h b kt", h=2))

    # keep_scal[p, b*32 + kt] = keeph[p // 64, b*32 + kt] via K=2 matmul
    selh = const.tile([2, 128], F16)
    nc.sync.dma_start(selh[:], selh_d)
    ks_ps = psum.tile([128, 128], F32, tag="ks_ps")
    nc.tensor.matmul(ks_ps[:], lhsT=selh[:], rhs=keeph[:], start=True, stop=True)
    keep_scal = mk.tile([128, 128], F16)
    nc.scalar.activation(keep_scal[:], ks_ps[:], ACT.Copy)

    # ---- x transposed for the GEMM: xt[p, kt*128 + m] = x[m, 128 kt + p] ----
    xt = xbuf.tile([128, K], F16)
    for kt in range(NKT):
        nc.sync.dma_start_transpose(xt[:, kt * 128:(kt + 1) * 128],
                                    x_d[:, kt * 128:(kt + 1) * 128])

    # masked xT, per k tile: xm[p, kt, b, m] = xt * keep[2kt + p//64, b]
    xm = xbuf.tile([128, K], F16)
    for kt in range(NKT):
        nc.vector.tensor_tensor(
            xm[:, kt * 128:(kt + 1) * 128].rearrange("p (b m) -> p b m", b=NBM),
            xt[:, kt * 128:(kt + 1) * 128].rearrange("p (b m) -> p b m", b=NBM),
            keep_scal[:, kt:kt + 97:32].unsqueeze(-1).broadcast_to((128, NBM, BLOCK_M)),
            op=ALU.mult)

    # ---- bias row and ones column for the rank-1 bias accumulation ----
    bias_sb = const.tile([1, NLOC], F16)
    nc.sync.dma_start(bias_sb[:], b_d)
    ones = const.tile([1, 128], F16)
    nc.vector.memset(ones[:], 1.0)

    # ---- main GEMM: out[m, n] = sum_kt xm_kt.T @ w_kt + ones.T @ bias ----
    pbanks = [psum.tile([128, 512], F32, name=f"pn{i}", tag=f"pn{i}")
              for i in range(3)]
    for kt in range(NKT):
        w_t = wpool.tile([128, NLOC], F16)
        nc.sync.dma_start(w_t[:], w_d[kt * 128:(kt + 1) * 128, :])
        for nt, (n0, nsz) in enumerate(N_TILES):
            nc.tensor.matmul(pbanks[nt][:, :nsz],
                             lhsT=xm[:, kt * 128:(kt + 1) * 128],
                             rhs=w_t[:, n0:n0 + nsz],
                             start=(kt == 0), stop=False)
    out_sb = opool.tile([128, NLOC], F16)
    for nt, (n0, nsz) in enumerate(N_TILES):
        nc.tensor.matmul(pbanks[nt][:, :nsz], lhsT=ones[:],
                         rhs=bias_sb[:, n0:n0 + nsz], start=False, stop=True)
        nc.vector.tensor_copy(out_sb[:, n0:n0 + nsz], pbanks[nt][:, :nsz])
        nc.sync.dma_start(o_d[:, n0:n0 + nsz], out_sb[:, n0:n0 + nsz])


_CACHE = {}


def _build():
    if "nc" in _CACHE:
        return _CACHE["nc"]
    nc = bacc.Bacc("TRN2", target_bir_lowering=False, debug=False,
                   num_devices=NCORES)
    x_d = nc.dram_tensor("x", (M, K), F16, kind="ExternalInput").ap()
    w_d = nc.dram_tensor("w", (K, NLOC), F16, kind="ExternalInput").ap()
    b_d = nc.dram_tensor("bias", (1, NLOC), F16, kind="ExternalInput").ap()
    e_d = nc.dram_tensor("E", (M, NBM), F32, kind="ExternalInput").ap()
    selh_d = nc.dram_tensor("selh", (2, 128), F16, kind="ExternalInput").ap()
    o_d = nc.dram_tensor("out", (M, NLOC), F16, kind="ExternalOutput").ap()
    with tile.TileContext(nc) as tc:
        with ExitStack() as ctx:
            _program(ctx, tc, [x_d, w_d, b_d, e_d, selh_d], [o_d])
    nc.compile()
    _CACHE["nc"] = nc
    return nc


def _make_in_maps(x2, weight, bias):
    e_np = np.zeros((M, NBM), np.float32)
    for b in range(NBM):
        e_np[b * BLOCK_M:(b + 1) * BLOCK_M, b] = 1.0
    selh_np = np.zeros((2, 128), np.float16)
    selh_np[0, 0:64] = 1.0
    selh_np[1, 64:128] = 1.0

    in_maps = []
    for c in range(NCORES):
        sl = slice(c * NLOC, (c + 1) * NLOC)
        in_maps.append({
            "x": x2,
            "w": np.ascontiguousarray(weight[:, sl].astype(np.float16, copy=False)),
            "bias": np.ascontiguousarray(
                np.asarray(bias)[sl].astype(np.float16, copy=False).reshape(1, NLOC)),
            "E": e_np,
            "selh": selh_np,
        })
    return in_maps


def kernel(x: np.ndarray, weight: np.ndarray, bias: np.ndarray) -> np.ndarray:
    x = np.asarray(x)
    weight = np.asarray(weight)
    bias = np.asarray(bias)
    bsz, seq, hidden = x.shape
    assert (bsz, seq, hidden) == (M, 1, K) and weight.shape == (K, N)

    x2 = np.ascontiguousarray(x.reshape(M, K).astype(np.float16, copy=False))
    in_maps = _make_in_maps(x2, weight, bias)
    nc = _build()
    res = run_bass_kernel_spmd(nc, in_maps, core_ids=list(range(NCORES)))
    out = np.concatenate([r["out"] for r in res.results], axis=1)
    return out.reshape(M, 1, N).astype(x.dtype, copy=False)


if __name__ == "__main__":
    rng = np.random.default_rng(0)
    x = rng.standard_normal((M, 1, K)).astype(np.float16)
    w = (rng.standard_normal((K, N)) * 0.01).astype(np.float16)
    b = np.zeros((N,), np.float16)
    out = kernel(x, w, b)
    print(out.shape, out.dtype)


# revision 6
# speedup vs baseline: 1.3306x; 1.3306x over previous
"""Block-sparse top-k masked linear for Trainium2, tensor-parallel over 8 cores.

out = (block_masked x) @ W + bias
  x: (128, 1, 4096) fp16, W: (4096, 11008) fp16, bias: (11008,) fp16
  mask: per (32-row x 64-col) block of x, keep blocks whose mean |x| is
  >= the 32nd-largest of the 64 k-block activations in that row block.

Sharding: column-parallel — each of the 8 cores gets an 11008/8 = 1376
column slice of W and bias; x is replicated; outputs are concatenated.
"""
from contextlib import ExitStack

import numpy as np

import concourse.bass as bass
import concourse.tile as tile
from concourse import bacc, mybir
from concourse.bass_utils import run_bass_kernel_spmd

F16 = mybir.dt.float16
F32 = mybir.dt.float32
AX = mybir.AxisListType
ALU = mybir.AluOpType
ACT = mybir.ActivationFunctionType

M = 128          # rows of x
K = 4096         # contraction
N = 11008        # out features
NCORES = 8
NLOC = N // NCORES           # 1376 columns per core
BLOCK_M, BLOCK_K = 32, 64
NBM, NBK = M // BLOCK_M, K // BLOCK_K   # 4 row blocks, 64 k blocks
KEEP = 32                               # k blocks kept per row block
NKT = K // 128                          # 32 k tiles of 128
N_TILES = [(0, 512), (512, 512), (1024, 352)]   # n-tile offsets/sizes
W_BUFS = 6


def _program(ctx: ExitStack, tc: tile.TileContext, ins, outs):
    nc = tc.nc
    x_d, w_d, b_d, e_d, selh_d = ins
    (o_d,) = outs

    const = ctx.enter_context(tc.tile_pool(name="const", bufs=1))
    xbuf = ctx.enter_context(tc.tile_pool(name="xbuf", bufs=1))
    mk = ctx.enter_context(tc.tile_pool(name="mk", bufs=1))
    wpool = ctx.enter_context(tc.tile_pool(name="wpool", bufs=W_BUFS))
    opool = ctx.enter_context(tc.tile_pool(name="opool", bufs=1))
    psum = ctx.enter_context(tc.tile_pool(name="psum", bufs=1, space="PSUM"))
    dram = ctx.enter_context(tc.tile_pool(name="dram", bufs=1, space="DRAM"))

    # ---- mask pipeline (natural x layout; independent of the transposes) ----
    # x_sb[m, k], loaded in 4 column chunks so the reduce can start early
    x_sb = xbuf.tile([128, K], F16)
    part_n = mk.tile([128, NBK], F32)
    for c in range(4):
        ks = K // 4
        nc.scalar.dma_start(x_sb[:, c * ks:(c + 1) * ks], x_d[:, c * ks:(c + 1) * ks])
        # part_n[m, j] = sum_k |x[m, 64 j + k]| over the 16 j's in this chunk
        nc.vector.tensor_reduce(
            part_n[:, c * 16:(c + 1) * 16],
            x_sb[:, c * ks:(c + 1) * ks].rearrange("p (j k) -> p j k", k=BLOCK_K),
            axis=AX.X, op=ALU.add, apply_absolute_value=True)

    # ba_ps[b, j] = sum_m E[m, b] * part_n[m, j]  (block sums, b on partitions)
    e_sb = const.tile([128, NBM], F32)
    nc.sync.dma_start(e_sb[:], e_d)
    ba_ps = psum.tile([NBM, NBK], F32, tag="ba_ps")
    nc.tensor.matmul(ba_ps[:], lhsT=e_sb[:], rhs=part_n[:], start=True, stop=True)

    # mean = sum / 2048 (exact power of two), rounded to f16 like jnp.mean
    ba16 = mk.tile([NBM, NBK], F16)
    nc.vector.tensor_scalar_mul(ba16[:], ba_ps[:], 1.0 / 2048.0)

    # DRAM round trip #1: replicate/transpose the 4x64 activations
    ba_dram = dram.tile([NBM, NBK], F16)
    nc.sync.dma_start(ba_dram[:], ba16[:])
    # arow[i, b*64 + j] = a[b, j]  (same 256 values on all 64 partitions)
    arow = mk.tile([64, NBM * NBK], F16)
    nc.sync.dma_start(
        arow[:].rearrange("i (b j) -> i b j", b=NBM),
        ba_dram[:].unsqueeze(0).broadcast_to((64, NBM, NBK)))
    # acol[i, b] = a[b, i]
    acol = mk.tile([64, NBM], F16)
    nc.sync.dma_start(acol[:], ba_dram[:].rearrange("b j -> j b"))

    # cnt[i, b] = #{j : a[b, j] > a[b, i]};  keep iff cnt < KEEP
    cmp = mk.tile([64, NBM * NBK], F16)
    nc.vector.tensor_tensor(
        cmp[:].rearrange("i (b j) -> i b j", b=NBM),
        arow[:].rearrange("i (b j) -> i b j", b=NBM),
        acol[:].unsqueeze(-1).broadcast_to((64, NBM, NBK)),
        op=ALU.is_gt)
    cnt = mk.tile([64, NBM], F32)
    nc.vector.tensor_reduce(cnt[:], cmp[:].rearrange("i (b j) -> i b j", b=NBM),
                            axis=AX.X, op=ALU.add)
    keep16 = mk.tile([64, NBM], F16)
    nc.vector.tensor_scalar(keep16[:], cnt[:], float(KEEP), None, op0=ALU.is_lt)

    # DRAM round trip #2: keeph[h, b*32 + kt] = keep16[2 kt + h, b]
    keep_dram = dram.tile([64, NBM], F16)
    nc.sync.dma_start(keep_dram[:], keep16[:])
    keeph = mk.tile([2, 128], F16)
    nc.sync.dma_start(keeph[:].rearrange("h (b kt) -> h b kt", b=NBM),
                      keep_dram[:].rearrange("(kt h) b -> h b kt", h=2))

    # keep_scal[p, b*32 + kt] = keeph[p // 64, b*32 + kt] via K=2 matmul
    selh = const.tile([2, 128], F16)
    nc.sync.dma_start(selh[:], selh_d)
    ks_ps = psum.tile([128, 128], F32, tag="ks_ps")
    nc.tensor.matmul(ks_ps[:], lhsT=selh[:], rhs=keeph[:], start=True, stop=True)
    keep_scal = mk.tile([128, 128], F16)
    nc.vector.tensor_copy(keep_scal[:], ks_ps[:])

    # ---- x transposed for the GEMM: xt[p, kt*128 + m] = x[m, 128 kt + p] ----
    # one xbar-transpose call: out[p, kt, m] = in[m, kt*128 + p]
    xt = xbuf.tile([128, K], F16)
    nc.scalar.dma_start_transpose(
        xt[:].rearrange("p (kt m) -> p kt m", kt=NKT), x_d)

    # masked xT, per k tile: xm[p, kt, b, m] = xt * keep[2kt + p//64, b]
    xm = xbuf.tile([128, K], F16)
    for kt in range(NKT):
        nc.vector.tensor_tensor(
            xm[:, kt * 128:(kt + 1) * 128].rearrange("p (b m) -> p b m", b=NBM),
            xt[:, kt * 128:(kt + 1) * 128].rearrange("p (b m) -> p b m", b=NBM),
            keep_scal[:, kt:kt + 97:32].unsqueeze(-1).broadcast_to((128, NBM, BLOCK_M)),
            op=ALU.mult)

    # ---- bias row and ones column for the rank-1 bias accumulation ----
    bias_sb = const.tile([1, NLOC], F16)
    nc.sync.dma_start(bias_sb[:], b_d)
    ones = const.tile([1, 128], F16)
    nc.vector.memset(ones[:], 1.0)

    # ---- main GEMM: out[m, n] = sum_kt xm_kt.T @ w_kt + ones.T @ bias ----
    pbanks = [psum.tile([128, 512], F32, name=f"pn{i}", tag=f"pn{i}")
              for i in range(3)]
    for kt in range(NKT):
        w_t = wpool.tile([128, NLOC], F16)
        eng = nc.sync if kt % 2 == 0 else nc.gpsimd
        eng.dma_start(w_t[:], w_d[kt * 128:(kt + 1) * 128, :])
        for nt, (n0, nsz) in enumerate(N_TILES):
            nc.tensor.matmul(pbanks[nt][:, :nsz],
                             lhsT=xm[:, kt * 128:(kt + 1) * 128],
                             rhs=w_t[:, n0:n0 + nsz],
                             start=(kt == 0), stop=False)
    out_sb = opool.tile([128, NLOC], F16)
    for nt, (n0, nsz) in enumerate(N_TILES):
        nc.tensor.matmul(pbanks[nt][:, :nsz], lhsT=ones[:],
                         rhs=bias_sb[:, n0:n0 + nsz], start=False, stop=True)
        nc.vector.tensor_copy(out_sb[:, n0:n0 + nsz], pbanks[nt][:, :nsz])
        nc.sync.dma_start(o_d[:, n0:n0 + nsz], out_sb[:, n0:n0 + nsz])


_CACHE = {}


def _build():
    if "nc" in _CACHE:
        return _CACHE["nc"]
    nc = bacc.Bacc("TRN2", target_bir_lowering=False, debug=False,
                   num_devices=NCORES)
    x_d = nc.dram_tensor("x", (M, K), F16, kind="ExternalInput").ap()
    w_d = nc.dram_tensor("w", (K, NLOC), F16, kind="ExternalInput").ap()
    b_d = nc.dram_tensor("bias", (1, NLOC), F16, kind="ExternalInput").ap()
    e_d = nc.dram_tensor("E", (M, NBM), F32, kind="ExternalInput").ap()
    selh_d = nc.dram_tensor("selh", (2, 128), F16, kind="ExternalInput").ap()
    o_d = nc.dram_tensor("out", (M, NLOC), F16, kind="ExternalOutput").ap()
    with tile.TileContext(nc) as tc:
        with ExitStack() as ctx:
            _program(ctx, tc, [x_d, w_d, b_d, e_d, selh_d], [o_d])
    nc.compile()
    _CACHE["nc"] = nc
    return nc


def _make_in_maps(x2, weight, bias):
    e_np = np.zeros((M, NBM), np.float32)
    for b in range(NBM):
        e_np[b * BLOCK_M:(b + 1) * BLOCK_M, b] = 1.0
    selh_np = np.zeros((2, 128), np.float16)
    selh_np[0, 0:64] = 1.0
    selh_np[1, 64:128] = 1.0

    in_maps = []
    for c in range(NCORES):
        sl = slice(c * NLOC, (c + 1) * NLOC)
        in_maps.append({
            "x": x2,
            "w": np.ascontiguousarray(weight[:, sl].astype(np.float16, copy=False)),
            "bias": np.ascontiguousarray(
                np.asarray(bias)[sl].astype(np.float16, copy=False).reshape(1, NLOC)),
            "E": e_np,
            "selh": selh_np,
        })
    return in_maps


def kernel(x: np.ndarray, weight: np.ndarray, bias: np.ndarray) -> np.ndarray:
    x = np.asarray(x)
    weight = np.asarray(weight)
    bias = np.asarray(bias)
    bsz, seq, hidden = x.shape
    assert (bsz, seq, hidden) == (M, 1, K) and weight.shape == (K, N)

    x2 = np.ascontiguousarray(x.reshape(M, K).astype(np.float16, copy=False))
    in_maps = _make_in_maps(x2, weight, bias)
    nc = _build()
    res = run_bass_kernel_spmd(nc, in_maps, core_ids=list(range(NCORES)))
    out = np.concatenate([r["out"] for r in res.results], axis=1)
    return out.reshape(M, 1, N).astype(x.dtype, copy=False)


if __name__ == "__main__":
    rng = np.random.default_rng(0)
    x = rng.standard_normal((M, 1, K)).astype(np.float16)
    w = (rng.standard_normal((K, N)) * 0.01).astype(np.float16)
    b = np.zeros((N,), np.float16)
    out = kernel(x, w, b)
    print(out.shape, out.dtype)


# revision 11
# speedup vs baseline: 1.6166x; 1.2149x over previous
"""Block-sparse top-k masked linear for Trainium2, tensor-parallel over 8 cores.

out = (block_masked x) @ W + bias
  x: (128, 1, 4096) fp16, W: (4096, 11008) fp16, bias: (11008,) fp16
  mask: per (32-row x 64-col) block of x, keep blocks whose mean |x| is
  >= the 32nd-largest of the 64 k-block activations in that row block.

Sharding: column-parallel — each of the 8 cores gets an 11008/8 = 1376
column slice of W and bias; x is replicated; outputs are concatenated.
"""
from contextlib import ExitStack

import numpy as np

import concourse.bass as bass
import concourse.tile as tile
from concourse import bacc, mybir
from concourse.bass_utils import run_bass_kernel_spmd

F16 = mybir.dt.float16
F32 = mybir.dt.float32
AX = mybir.AxisListType
ALU = mybir.AluOpType
ACT = mybir.ActivationFunctionType

M = 128          # rows of x
K = 4096         # contraction
N = 11008        # out features
NCORES = 8
NLOC = N // NCORES           # 1376 columns per core
BLOCK_M, BLOCK_K = 32, 64
NBM, NBK = M // BLOCK_M, K // BLOCK_K   # 4 row blocks, 64 k blocks
KEEP = 32                               # k blocks kept per row block
NKT = K // 128                          # 32 k tiles of 128
N_TILES = [(0, 512), (512, 512), (1024, 352)]   # n-tile offsets/sizes
W_BUFS = 12


def _program(ctx: ExitStack, tc: tile.TileContext, ins, outs):
    nc = tc.nc
    x_d, w_d, b_d, e_d, id_d, jh_d, ksel_d = ins
    (o_d,) = outs

    const = ctx.enter_context(tc.tile_pool(name="const", bufs=1))
    xbuf = ctx.enter_context(tc.tile_pool(name="xbuf", bufs=1))
    mk = ctx.enter_context(tc.tile_pool(name="mk", bufs=1))
    wpool = ctx.enter_context(tc.tile_pool(name="wpool", bufs=W_BUFS))
    opool = ctx.enter_context(tc.tile_pool(name="opool", bufs=1))
    psum = ctx.enter_context(tc.tile_pool(name="psum", bufs=1, space="PSUM"))

    # ---- mask pipeline (natural x layout; independent of the transpose) ----
    # x_sb[m, k], loaded in 8 column chunks (on sync) so the reduce starts early
    NCH = 8
    x_sb = xbuf.tile([128, K], F16)
    part_n = mk.tile([128, NBK], F32)
    jc = NBK // NCH
    for c in range(NCH):
        ks = K // NCH
        nc.sync.dma_start(x_sb[:, c * ks:(c + 1) * ks], x_d[:, c * ks:(c + 1) * ks])
        # part_n[m, j] = sum_k |x[m, 64 j + k]| over this chunk's j's
        nc.vector.tensor_reduce(
            part_n[:, c * jc:(c + 1) * jc],
            x_sb[:, c * ks:(c + 1) * ks].rearrange("p (j k) -> p j k", k=BLOCK_K),
            axis=AX.X, op=ALU.add, apply_absolute_value=True)

    # consts on the scalar ring (scalar carries no weights)
    e_sb = const.tile([128, NBM], F32)
    nc.scalar.dma_start(e_sb[:], e_d)
    ident = const.tile([128, 128], F16)
    nc.scalar.dma_start(ident[:], id_d)
    jh = const.tile([64, 128], F16)
    nc.scalar.dma_start(jh[:], jh_d)
    ksel = const.tile([64, NKT], F16)
    nc.scalar.dma_start(ksel[:], ksel_d)
    bias_sb = const.tile([1, NLOC], F16)
    nc.scalar.dma_start(bias_sb[:], b_d)

    # ---- x transposed for the GEMM: xt[p, kt*128 + m] = x[m, 128 kt + p] ----
    # one xbar-transpose call on the scalar ring: out[p, kt, m] = in[m, kt*128+p]
    xt = xbuf.tile([128, K], F16)
    nc.scalar.dma_start_transpose(
        xt[:].rearrange("p (kt m) -> p kt m", kt=NKT), x_d)

    # ba_ps[b, j] = sum_m E[m, b] * part_n[m, j]  (block sums, b on partitions)
    ba_ps = psum.tile([NBM, NBK], F32, tag="ba_ps")
    nc.tensor.matmul(ba_ps[:], lhsT=e_sb[:], rhs=part_n[:], start=True, stop=True)

    # mean = sum / 2048 (exact power of two), rounded to f16 like jnp.mean
    ba16 = mk.tile([NBM, NBK], F16)
    nc.vector.tensor_scalar_mul(ba16[:], ba_ps[:], 1.0 / 2048.0)

    # arow[i, b*64+j] = a[b, j] on 64 partitions, via block-diag expand + matmul
    # rhs3[c, b*64+j] = a[c, j] * [c == b]
    rhs3 = mk.tile([NBM, NBM * NBK], F16)
    nc.vector.tensor_tensor(
        rhs3[:].rearrange("c (b j) -> c b j", b=NBM),
        ba16[:].unsqueeze(1).broadcast_to((NBM, NBM, NBK)),
        ident[0:NBM, 0:NBM].unsqueeze(-1).broadcast_to((NBM, NBM, NBK)),
        op=ALU.mult)
    ones4c = mk.tile([NBM, 64], F16)
    nc.vector.memset(ones4c[:], 1.0)
    arow_ps = psum.tile([64, NBM * NBK], F32, tag="arow_ps")
    nc.tensor.matmul(arow_ps[:], lhsT=ones4c[:], rhs=rhs3[:], start=True, stop=True)
    arow = mk.tile([64, NBM * NBK], F16)
    nc.vector.tensor_copy(arow[:], arow_ps[:])

    # acol[i, b] = a[b, i] via PE transpose
    acol_ps = psum.tile([64, NBM], F16, tag="acol_ps")
    nc.tensor.transpose(acol_ps[:], ba16[:], ident[0:NBM, 0:NBM])
    acol = mk.tile([64, NBM], F16)
    nc.vector.tensor_copy(acol[:], acol_ps[:])

    # cnt[i, b] = #{j : a[b, j] > a[b, i]};  keep iff cnt < KEEP
    cmp = mk.tile([64, NBM * NBK], F16)
    nc.vector.tensor_tensor(
        cmp[:].rearrange("i (b j) -> i b j", b=NBM),
        arow[:].rearrange("i (b j) -> i b j", b=NBM),
        acol[:].unsqueeze(-1).broadcast_to((64, NBM, NBK)),
        op=ALU.is_gt)
    cnt = mk.tile([64, NBM], F32)
    nc.vector.tensor_reduce(cnt[:], cmp[:].rearrange("i (b j) -> i b j", b=NBM),
                            axis=AX.X, op=ALU.add)
    keep16 = mk.tile([64, NBM], F16)
    nc.vector.tensor_scalar(keep16[:], cnt[:], float(KEEP), None, op0=ALU.is_lt)

    # keep_scal[p, b*32+kt] = keep16[2kt + p//64, b]
    #   = sum_j [j%2 == p//64] * keep16[j, b] * [j//2 == kt]  (factored selector)
    # rhs2[j, b*32+kt] = keep16[j, b] * Ksel[j, kt]
    rhs2 = mk.tile([64, 128], F16)
    nc.vector.tensor_tensor(
        rhs2[:].rearrange("j (b kt) -> j b kt", b=NBM),
        keep16[:].unsqueeze(-1).broadcast_to((64, NBM, NKT)),
        ksel[:].unsqueeze(1).broadcast_to((64, NBM, NKT)),
        op=ALU.mult)
    ks_ps = psum.tile([128, 128], F32, tag="ks_ps")
    nc.tensor.matmul(ks_ps[:], lhsT=jh[:], rhs=rhs2[:], start=True, stop=True)
    keep_scal = mk.tile([128, 128], F16)
    nc.vector.tensor_copy(keep_scal[:], ks_ps[:])

    ones = const.tile([1, 128], F16)
    nc.vector.memset(ones[:], 1.0)

    # ---- main GEMM: out[m, n] = sum_kt xm_kt.T @ w_kt + ones.T @ bias ----
    xm = xbuf.tile([128, K], F16)
    pbanks = [psum.tile([128, 512], F32, name=f"pn{i}", tag=f"pn{i}")
              for i in range(3)]
    for kt in range(NKT):
        w_t = wpool.tile([128, NLOC], F16)
        eng = nc.sync if kt % 2 == 0 else nc.gpsimd
        eng.dma_start(w_t[:], w_d[kt * 128:(kt + 1) * 128, :])
        # masked xT for this k tile: xm[p, kt, b, m] = xt * keep[2kt+p//64, b]
        nc.vector.tensor_tensor(
            xm[:, kt * 128:(kt + 1) * 128].rearrange("p (b m) -> p b m", b=NBM),
            xt[:, kt * 128:(kt + 1) * 128].rearrange("p (b m) -> p b m", b=NBM),
            keep_scal[:, kt:kt + 97:32].unsqueeze(-1).broadcast_to((128, NBM, BLOCK_M)),
            op=ALU.mult)
        for nt, (n0, nsz) in enumerate(N_TILES):
            nc.tensor.matmul(pbanks[nt][:, :nsz],
                             lhsT=xm[:, kt * 128:(kt + 1) * 128],
                             rhs=w_t[:, n0:n0 + nsz],
                             start=(kt == 0), stop=False)
    out_sb = opool.tile([128, NLOC], F16)
    for nt, (n0, nsz) in enumerate(N_TILES):
        nc.tensor.matmul(pbanks[nt][:, :nsz], lhsT=ones[:],
                         rhs=bias_sb[:, n0:n0 + nsz], start=False, stop=True)
        nc.scalar.activation(out_sb[:, n0:n0 + nsz], pbanks[nt][:, :nsz], ACT.Copy)
        nc.scalar.dma_start(o_d[:, n0:n0 + nsz], out_sb[:, n0:n0 + nsz])


_CACHE = {}


def _build():
    if "nc" in _CACHE:
        return _CACHE["nc"]
    nc = bacc.Bacc("TRN2", target_bir_lowering=False, debug=False,
                   num_devices=NCORES)
    x_d = nc.dram_tensor("x", (M, K), F16, kind="ExternalInput").ap()
    w_d = nc.dram_tensor("w", (K, NLOC), F16, kind="ExternalInput").ap()
    b_d = nc.dram_tensor("bias", (1, NLOC), F16, kind="ExternalInput").ap()
    e_d = nc.dram_tensor("E", (M, NBM), F32, kind="ExternalInput").ap()
    id_d = nc.dram_tensor("ident", (128, 128), F16, kind="ExternalInput").ap()
    jh_d = nc.dram_tensor("JH", (64, 128), F16, kind="ExternalInput").ap()
    ksel_d = nc.dram_tensor("Ksel", (64, NKT), F16, kind="ExternalInput").ap()
    o_d = nc.dram_tensor("out", (M, NLOC), F16, kind="ExternalOutput").ap()
    with tile.TileContext(nc) as tc:
        with ExitStack() as ctx:
            _program(ctx, tc, [x_d, w_d, b_d, e_d, id_d, jh_d, ksel_d], [o_d])
    nc.compile()
    _CACHE["nc"] = nc
    return nc


def _make_in_maps(x2, weight, bias):
    e_np = np.zeros((M, NBM), np.float32)
    for b in range(NBM):
        e_np[b * BLOCK_M:(b + 1) * BLOCK_M, b] = 1.0
    id_np = np.eye(128, dtype=np.float16)
    j_idx = np.arange(64)
    jh_np = (j_idx[:, None] % 2 == (np.arange(128)[None, :] // 64)).astype(np.float16)
    ksel_np = (j_idx[:, None] // 2 == np.arange(NKT)[None, :]).astype(np.float16)

    in_maps = []
    for c in range(NCORES):
        sl = slice(c * NLOC, (c + 1) * NLOC)
        in_maps.append({
            "x": x2,
            "w": np.ascontiguousarray(weight[:, sl].astype(np.float16, copy=False)),
            "bias": np.ascontiguousarray(
                np.asarray(bias)[sl].astype(np.float16, copy=False).reshape(1, NLOC)),
            "E": e_np,
            "ident": id_np,
            "JH": jh_np,
            "Ksel": ksel_np,
        })
    return in_maps


def kernel(x: np.ndarray, weight: np.ndarray, bias: np.ndarray) -> np.ndarray:
    x = np.asarray(x)
    weight = np.asarray(weight)
    bias = np.asarray(bias)
    bsz, seq, hidden = x.shape
    assert (bsz, seq, hidden) == (M, 1, K) and weight.shape == (K, N)

    x2 = np.ascontiguousarray(x.reshape(M, K).astype(np.float16, copy=False))
    in_maps = _make_in_maps(x2, weight, bias)
    nc = _build()
    res = run_bass_kernel_spmd(nc, in_maps, core_ids=list(range(NCORES)))
    out = np.concatenate([r["out"] for r in res.results], axis=1)
    return out.reshape(M, 1, N).astype(x.dtype, copy=False)


if __name__ == "__main__":
    rng = np.random.default_rng(0)
    x = rng.standard_normal((M, 1, K)).astype(np.float16)
    w = (rng.standard_normal((K, N)) * 0.01).astype(np.float16)
    b = np.zeros((N,), np.float16)
    out = kernel(x, w, b)
    print(out.shape, out.dtype)
